# revision 1
# baseline (speedup 1.0000x reference)
"""Trainium2 Bass kernel for nn_Attention_47467978555850.

Multi-head attention (B=8, N=1024, E=768, H=12, D=64), fp32.
Sharding: data-parallel over batch — one batch element per NeuronCore (8 cores),
no collectives.

Per-core dataflow (everything stays in "transposed" space so no on-device
transposes are needed; the host transposes x and y, which costs no HW time):

  xT [E, N]  --(w_qkv lhsT-stationary)-->  qT, kT  [head-dim major, N]
                                           (2 heads packed per 128-partition tile)
  xT (stationary) x w_v (moving)  ->  v [N, d] -> v_aug [N, H*128], each head
                                      block = [v(64) | ones(64)]
  For each head pair (2f, 2f+1), for each context tile j:
    S^T[j,i] both heads     : row-packed K=64 matmuls (head A in array rows
                              0-63 -> psum bank c, head B rows 64-127 ->
                              other bank) — the two heads run concurrently
    expS^T = Exp(S^T * 1/8) : one [128,1024] ScalarE op per (j, i-chunk);
                              no max-subtraction (scores ~N(0, 0.31), exp
                              range ~[0.1, 10], no overflow possible)
    out_aug^T += v_aug^T @ expS^T : psum rows 0-63 = out, rows 64-127 = the
                              softmax denominator replicated 64x (the ones
                              block makes the matmul broadcast it for free)
  outT = out_aug^T[0:64] * reciprocal(out_aug^T[64:128])   (pure DVE, 64 lanes)
  yT = w_proj^T @ outT + b  ->  DMA out as yT [E, N]

All matmuls run as float32r (fp32 storage, ~1 cycle/row PE streaming for
moving free-dim >= 256). Measured end-to-end ~330 us/core on HW,
absmax-relative error 1.6e-04 vs fp64.
"""

import numpy as np

B, N, E = 8, 1024, 768
H, D = 12, 64
NE = E // 128        # 6  e-tiles
NT = N // 128        # 8  token tiles
JT = N // 128        # 8  j tiles (attention context)
CH = N // 512        # 2  512-wide moving chunks
DA = 2 * D           # 128 cols/head in v_aug: [v(64), ones(64)] — the
                     # ones block makes mm3 replicate the softmax denom
                     # across 64 psum partitions (free: matmul cost ~ N)

_NC_CACHE = {}

# Timing-experiment switch (leave "full" for real runs):
#   full  - everything
#   nomm3 - skip attn@v matmuls + normalization
#   noexp - also skip exp (attention = scores matmuls only)
#   qkv   - skip attention entirely (v + qk + proj only)
VARIANT = "full"


def _emit(tc, pools, aps):
    import concourse.mybir as mybir

    nc = tc.nc
    f32 = mybir.dt.float32
    f32r = mybir.dt.float32r
    consts, wstr, expp, qkp, rbp, ytp, scr, psu, psacc = pools
    xT, w_qkv, w_proj, b_proj, yT = aps

    # ---- persistent SBUF tiles ----
    xt = [consts.tile([128, N], f32r, tag=f"xt{e}", name=f"xt{e}") for e in range(NE)]
    wv = [consts.tile([128, E], f32r, tag=f"wv{e}", name=f"wv{e}") for e in range(NE)]
    b_sb = consts.tile([128, NE], f32, tag="b_sb", name="b_sb")
    vaug = [consts.tile([128, H * DA], f32r, tag=f"va{t}", name=f"va{t}")
            for t in range(NT)]
    outT = [consts.tile([128, N], f32r, tag=f"oT{e}", name=f"oT{e}") for e in range(NE)]

    # chunk-split loads: the first qk matmul only needs xt[0][:, 0:512], so it
    # is gated by one 256 KB DMA instead of the whole 3 MB of xT
    for e in range(NE):
        nc.sync.dma_start(out=xt[e][:, 0:512],
                          in_=xT[e * 128:(e + 1) * 128, 0:512].bitcast(f32r))
    for e in range(NE):
        nc.sync.dma_start(out=xt[e][:, 512:N],
                          in_=xT[e * 128:(e + 1) * 128, 512:N].bitcast(f32r))
    for e in range(NE):
        nc.sync.dma_start(out=wv[e][:, 0:512],
                          in_=w_qkv[e * 128:(e + 1) * 128, 2 * E:2 * E + 512].bitcast(f32r))
    for e in range(NE):
        nc.sync.dma_start(out=wv[e][:, 512:E],
                          in_=w_qkv[e * 128:(e + 1) * 128, 2 * E + 512:3 * E].bitcast(f32r))
    nc.sync.dma_start(out=b_sb, in_=b_proj.rearrange("(t p) -> p t", p=128))
    ones_sb = consts.tile([128, 1], f32, tag="ones", name="ones_sb")
    nc.vector.memset(ones_sb, 1.0)

    # ---- phase 1: v = x @ w_v  (xT tiles stationary, w_v moving) ----
    # Emitted after the first qk feat-tiles: the qk matmuls only need small
    # weight tiles + xT, so they hide the 7 MB w_v DMA.
    def emit_v_phase():
      for t in range(NT):
          ps_v = psu.tile([128, N], f32, tag="ps", name=f"psv{t}")
          # chunk outer: 6-deep same-psum-bank accumulation runs (bank
          # changes cost more than the weight reloads they trade against)
          for (c0, cl) in ((0, 512), (512, 256)):
              for e in range(NE):
                  nc.tensor.matmul(
                      out=ps_v[:, c0:c0 + cl],
                      lhsT=(xt[e][:, t * 128:(t + 1) * 128]),
                      rhs=(wv[e][:, c0:c0 + cl]),
                      start=(e == 0), stop=(e == NE - 1),
                  )
          va3 = vaug[t].rearrange("p (h c) -> p h c", h=H)
          nc.vector.tensor_copy(
              out=va3[:, :, 0:D],
              in_=ps_v[:, 0:E].rearrange("p (h c) -> p h c", h=H),
          )
          nc.vector.tensor_copy(out=va3[:, :, D:DA],
                                in_=ones_sb.broadcast_to([128, H, D]))

    # ---- phase 2: per head-pair f: compute qT[f], kT[f], then attention ----
    def qk_feat_tile(fcol, fname):
        ps_qk = psu.tile([128, N], f32, tag="ps", name=f"psqk{fname}")
        wts = []
        for e in range(NE):
            w = wstr.tile([128, 128], f32r, tag="w", name=f"w{fname}e{e}")
            nc.sync.dma_start(
                out=w, in_=w_qkv[e * 128:(e + 1) * 128, fcol:fcol + 128].bitcast(f32r))
            wts.append(w)
        for c in range(CH):
            cs = slice(c * 512, (c + 1) * 512)
            for e in range(NE):
                nc.tensor.matmul(
                    out=ps_qk[:, cs], lhsT=(wts[e]), rhs=(xt[e][:, cs]),
                    start=(e == 0), stop=(e == NE - 1),
                )
        dst = qkp.tile([128, N], f32r, tag="qk", name=f"qk{fname}")
        nc.vector.tensor_copy(out=dst, in_=ps_qk)
        return dst

    def attention_pair(f, qTf, kTf):
        """Heads hA=2f (partitions 0:64 of qTf/kTf), hB=2f+1 (64:128).

        Two sequential phases, one per 512-wide i-chunk; 1-bank accumulators
        leave three floating [128, 1024] PSUM slots for the S pipeline.
        Within a phase, all 8 mm2+exp steps run first (E tiles pinned, expp
        bufs=10), then head A's 8-deep accumulation chain, then head B's —
        consecutive accumulating matmuls into the SAME psum bank are ~300 ns
        cheaper than bank-alternating ones (974 -> 678 ns/mm measured).
        mm2 stays row-packed (two heads concurrent in disjoint row groups).
        """
        hA, hB = 2 * f, 2 * f + 1

        for c in range(CH):
            cs = slice(c * 512, (c + 1) * 512)
            accA = accB = None
            if VARIANT == "full":
                accA = psacc.tile([128, 512], f32, tag="acc", name=f"accA{f}_{c}")
                accB = psacc.tile([128, 512], f32, tag="acc", name=f"accB{f}_{c}")

            def mm2exp(j):
                js = slice(j * 128, (j + 1) * 128)
                S = psu.tile([128, N], f32, tag="ps", name=f"S{f}_{c}_{j}")
                for pb, col0 in ((0, 0), (64, 512)):
                    nc.tensor.matmul(
                        out=S[:, col0:col0 + 512],
                        lhsT=kTf[pb:pb + 64, js],
                        rhs=qTf[pb:pb + 64, cs],
                        start=True, stop=True,
                    )
                if VARIANT == "noexp":
                    return None
                Ej = expp.tile([128, N], f32r, tag="e", name=f"E{f}_{c}_{j}")
                nc.scalar.activation(
                    out=Ej, in_=S,
                    func=mybir.ActivationFunctionType.Exp, scale=0.125)
                return Ej

            E_cur = mm2exp(0)
            for j in range(JT):
                E_next = mm2exp(j + 1) if j + 1 < JT else None
                if VARIANT in ("noexp", "nomm3"):
                    E_cur = E_next
                    continue
                for acc, col0, h in ((accA, 0, hA), (accB, 512, hB)):
                    nc.tensor.matmul(
                        out=acc,
                        lhsT=(vaug[j][:, h * DA:(h + 1) * DA]),
                        rhs=(E_cur[:, col0:col0 + 512]),
                        start=(j == 0), stop=(j == JT - 1),
                    )
                E_cur = E_next
            if VARIANT in ("noexp", "nomm3"):
                continue

            for acc, h in ((accA, hA), (accB, hB)):
                pb = (h % 2) * 64
                rb = rbp.tile([128, N], f32, tag="rb", name=f"rb{h}")
                nc.vector.reciprocal(out=rb[0:64, 0:512], in_=acc[64:128, :])
                nc.vector.tensor_mul(outT[f][pb:pb + 64, cs], acc[0:64, :],
                                     rb[0:64, 0:512])

    if VARIANT != "full":
        # keep outT written so the proj phase has valid producers
        for e in range(NE):
            nc.vector.tensor_copy(out=outT[e], in_=xt[e])
    for f in range(NE):
        qTf = qk_feat_tile(f * 128, f"q{f}")
        kTf = qk_feat_tile(E + f * 128, f"k{f}")
        if f == 0:
            emit_v_phase()
        if VARIANT != "qkv":
            attention_pair(f, qTf, kTf)

    # ---- phase 3: proj: yT = w_proj^T @ outT + b ----
    for g in range(NE):
        ps_y = psu.tile([128, N], f32, tag="ps", name=f"psy{g}")
        wts = []
        for e in range(NE):
            w = wstr.tile([128, 128], f32r, tag="w", name=f"wp{g}e{e}")
            nc.sync.dma_start(
                out=w, in_=w_proj[e * 128:(e + 1) * 128, g * 128:(g + 1) * 128].bitcast(f32r))
            wts.append(w)
        for c in range(CH):
            cs = slice(c * 512, (c + 1) * 512)
            for e in range(NE):
                nc.tensor.matmul(
                    out=ps_y[:, cs], lhsT=(wts[e]), rhs=(outT[e][:, cs]),
                    start=(e == 0), stop=(e == NE - 1),
                )
        yt = ytp.tile([128, N], f32, tag="yt", name=f"yt{g}")
        nc.vector.tensor_scalar_add(out=yt, in0=ps_y, scalar1=b_sb[:, g:g + 1])
        nc.sync.dma_start(out=yT[g * 128:(g + 1) * 128, :], in_=yt)


def build_nc(loop_n=1):
    """Build + compile the per-core Bass program. loop_n>1 wraps the body in a
    dynamic loop (used only for timing runs)."""
    from contextlib import ExitStack
    import concourse.bacc as bacc
    import concourse.mybir as mybir
    import concourse.tile as tile

    f32 = mybir.dt.float32
    nc = bacc.Bacc("TRN2", target_bir_lowering=False, debug=False)
    xT = nc.dram_tensor("xT", [E, N], f32, kind="ExternalInput").ap()
    w_qkv = nc.dram_tensor("w_qkv", [E, 3 * E], f32, kind="ExternalInput").ap()
    w_proj = nc.dram_tensor("w_proj", [E, E], f32, kind="ExternalInput").ap()
    b_proj = nc.dram_tensor("b_proj", [E], f32, kind="ExternalInput").ap()
    yT = nc.dram_tensor("yT", [E, N], f32, kind="ExternalOutput").ap()

    with tile.TileContext(nc) as tc, ExitStack() as ctx:
        pools = (
            ctx.enter_context(tc.tile_pool(name="consts", bufs=1)),
            ctx.enter_context(tc.tile_pool(name="wstr", bufs=12)),
            ctx.enter_context(tc.tile_pool(name="expp", bufs=6)),
            ctx.enter_context(tc.tile_pool(name="qkp", bufs=4)),
            ctx.enter_context(tc.tile_pool(name="rbp", bufs=2)),
            ctx.enter_context(tc.tile_pool(name="ytp", bufs=2)),
            ctx.enter_context(tc.tile_pool(name="scr", bufs=2)),
            ctx.enter_context(tc.tile_pool(name="psu", bufs=3, space="PSUM")),
            ctx.enter_context(tc.tile_pool(name="psacc", bufs=2, space="PSUM")),
        )
        aps = (xT, w_qkv, w_proj, b_proj, yT)
        if loop_n == 1:
            _emit(tc, pools, aps)
        else:
            # timing-only path; branch-prefetch hints avoid the per-iteration
            # IRAM refetch stall on the big-body engines
            with tc.For_i(0, loop_n, 1,
                          hint_engines=(mybir.EngineType.PE,
                                        mybir.EngineType.Activation,
                                        mybir.EngineType.DVE)):
                _emit(tc, pools, aps)
    nc.compile()
    return nc


def _get_nc(loop_n=1):
    if loop_n not in _NC_CACHE:
        _NC_CACHE[loop_n] = build_nc(loop_n)
    return _NC_CACHE[loop_n]


def kernel(x, w_qkv, w_proj, b_proj):
    """Full-input entry point: x [8,1024,768] f32 -> out [8,1024,768] f32."""
    from concourse.bass_utils import run_bass_kernel_spmd

    nc = _get_nc()
    x = np.asarray(x, dtype=np.float32)
    w_qkv = np.ascontiguousarray(np.asarray(w_qkv, dtype=np.float32))
    w_proj = np.ascontiguousarray(np.asarray(w_proj, dtype=np.float32))
    b_proj = np.ascontiguousarray(np.asarray(b_proj, dtype=np.float32))
    xT = np.ascontiguousarray(np.transpose(x, (0, 2, 1)))  # [B, E, N]
    in_maps = [
        {"xT": xT[c], "w_qkv": w_qkv, "w_proj": w_proj, "b_proj": b_proj}
        for c in range(B)
    ]
    res = run_bass_kernel_spmd(nc, in_maps, core_ids=list(range(B)))
    yT = np.stack([res.results[c]["yT"] for c in range(B)])  # [B, E, N]
    return np.ascontiguousarray(np.transpose(yT, (0, 2, 1)))



# revision 13
# speedup vs baseline: 1.4348x; 1.4348x over previous
"""Trainium2 Bass kernel for nn_Attention_47467978555850.

Multi-head attention (B=8, N=1024, E=768, H=12, D=64), fp32 in/out.
Sharding: data-parallel over batch - one batch element per NeuronCore (8 cores),
no collectives.

All matmul operands are fp16 (host-cast inputs; fp16 mantissa error 2^-11 is
below the fp32r matmul error of the original fp32 pipeline, and every value
here is far inside fp16 range). PSUM accumulation is fp32 throughout. fp16
also enables fast weight loads (FWL) and halves input DMA.

Per-core dataflow (everything stays in "transposed" space so no on-device
transposes are needed; the host transposes/casts x and y, costing no HW time):

  xTh [E, N] fp16 --(w_qkv lhsT-stationary)--> qT, kT fp16 [head-dim major, N]
              (2 heads packed per 128-partition tile)
  xTh (stationary) x w_v (moving) -> v [N, d] -> v_aug fp16 [N, H*128],
              each head block = [v(64) | ones(64)]
  For each head pair f = (2f, 2f+1), for each 512-wide i-chunk c:
    S^T[j,i] both heads : row-packed K=64 matmuls into the two banks of
              one [128,1024] PSUM tile (head A rows 0-63 -> cols 0:512, head B
              rows 64-127 -> cols 512:1024) - concurrent PE row groups
    E = Exp(S^T/8) fp16 : one [128,1024] ScalarE op per (c, j); ScalarE is the
              attention-phase critical engine (~1.07us/op, 96 ops)
    acc[:,0:512]   += vaugA^T @ E[:,0:512]    (8-deep fp16 chains; rows 0-63 =
    acc[:,512:1024]+= vaugB^T @ E[:,512:1024]  out, 64-127 = denominator)
    outT fp16 = acc[0:64] * reciprocal_approx_fast(denominator)
              (the custom DVE op breaks at base_partition 64, so the
              replicated denominator rows are first copied down to 0-63)
  yT = w_projh^T @ outT + b  ->  DMA out as yT [E, N] fp32

PSUM: ONE pool of 4x [128,1024] tiles (8 banks) shared by S / acc / qk-feat /
v / proj so the next pair's feature matmuls can fill PE slack while ScalarE
crunches exp. Next-pair q/k feat tiles are emitted between attention chunks.

Measured absmax-rel error vs fp64: ~4e-4 (gate 2e-2).
"""

import numpy as np
import ml_dtypes

B, N, E = 8, 1024, 768
H, D = 12, 64
NE = E // 128        # 6  e-tiles
NT = N // 128        # 8  token tiles
JT = N // 128        # 8  j tiles (attention context)
CH = N // 512        # 2  512-wide moving chunks
DA = 2 * D           # 128 cols/head in v_aug: [v(64), ones(64)] — the
                     # ones block makes the attn@v matmul replicate the
                     # softmax denominator across 64 psum partitions

_NC_CACHE = {}


def _emit(tc, pools, aps, dbg=None):
    import concourse.mybir as mybir

    nc = tc.nc
    f32 = mybir.dt.float32
    fp16 = mybir.dt.float16
    consts, wstr, expp, qkp, rbp, ytp, psu = pools
    xT, w_qkv, w_projh, b_proj, yT = aps

    def dump(name, src):
        # debug-only: copy an SBUF/PSUM AP out to a DRAM tensor
        if dbg is not None and name in dbg:
            nc.sync.dma_start(out=dbg[name], in_=src)

    def dump_psum(name, src, dt):
        if dbg is not None and name in dbg:
            t = consts.tile(list(src.shape), dt, tag=f"dbg{name}", name=f"dbg{name}")
            nc.vector.tensor_copy(out=t, in_=src)
            nc.sync.dma_start(out=dbg[name], in_=t)

    # ---- persistent SBUF tiles ----
    xt = [consts.tile([128, N], fp16, tag=f"xt{e}", name=f"xt{e}") for e in range(NE)]
    wv = [consts.tile([128, E], fp16, tag=f"wv{e}", name=f"wv{e}") for e in range(NE)]
    b_sb = consts.tile([128, NE], f32, tag="b_sb", name="b_sb")
    vaug = [consts.tile([128, H * DA], fp16, tag=f"va{t}", name=f"va{t}")
            for t in range(NT)]
    outT = [consts.tile([128, N], fp16, tag=f"oT{e}", name=f"oT{e}") for e in range(NE)]

    # chunk-split loads: the first qk matmul only needs xt[*][:, 0:512], so it
    # is gated by 1.5 MB of DMA instead of the whole 3 MB of xT
    for e in range(NE):
        nc.sync.dma_start(out=xt[e][:, 0:512],
                          in_=xT[e * 128:(e + 1) * 128, 0:512])
    for e in range(NE):
        nc.sync.dma_start(out=xt[e][:, 512:N],
                          in_=xT[e * 128:(e + 1) * 128, 512:N])
    for e in range(NE):
        nc.sync.dma_start(out=wv[e][:, 0:512],
                          in_=w_qkv[e * 128:(e + 1) * 128, 2 * E:2 * E + 512])
    for e in range(NE):
        nc.sync.dma_start(out=wv[e][:, 512:E],
                          in_=w_qkv[e * 128:(e + 1) * 128, 2 * E + 512:3 * E])
    nc.sync.dma_start(out=b_sb, in_=b_proj.rearrange("(t p) -> p t", p=128))

    # ---- q/k feature tiles: w_qkv column block stationary, xT moving ----
    def qk_feat_tile(fcol, fname):
        ps_qk = psu.tile([128, N], f32, tag="ps", name=f"psqk{fname}")
        wts = []
        for e in range(NE):
            w = wstr.tile([128, 128], fp16, tag="w", name=f"w{fname}e{e}")
            nc.sync.dma_start(
                out=w, in_=w_qkv[e * 128:(e + 1) * 128, fcol:fcol + 128])
            wts.append(w)
        for c in range(CH):
            cs = slice(c * 512, (c + 1) * 512)
            for e in range(NE):
                nc.tensor.matmul(
                    out=ps_qk[:, cs], lhsT=(wts[e]), rhs=(xt[e][:, cs]),
                    start=(e == 0), stop=(e == NE - 1),
                )
        dst = qkp.tile([128, N], fp16, tag="qk", name=f"qk{fname}")
        nc.vector.tensor_copy(out=dst, in_=ps_qk)
        return dst

    # ---- v = x @ w_v  (xT tiles stationary, w_v moving) -> vaug bf16 ----
    def emit_v_phase():
        for t in range(NT):
            ps_v = psu.tile([128, N], f32, tag="ps", name=f"psv{t}")
            for (c0, cl) in ((0, 512), (512, 256)):
                for e in range(NE):
                    nc.tensor.matmul(
                        out=ps_v[:, c0:c0 + cl],
                        lhsT=(xt[e][:, t * 128:(t + 1) * 128]),
                        rhs=(wv[e][:, c0:c0 + cl]),
                        start=(e == 0), stop=(e == NE - 1),
                    )
            va3 = vaug[t].rearrange("p (h c) -> p h c", h=H)
            nc.vector.tensor_copy(
                out=va3[:, :, 0:D],
                in_=ps_v[:, 0:E].rearrange("p (h c) -> p h c", h=H),
            )
            nc.vector.memset(va3[:, :, D:DA], 1.0)
            if t == 0:
                dump("vaug0", vaug[0])

    # ---- attention for head pair f, one 512-wide i-chunk ----
    def attention_chunk(f, c, qTf, kTf):
        hA, hB = 2 * f, 2 * f + 1
        cs = slice(c * 512, (c + 1) * 512)

        def mm2exp(j):
            js = slice(j * 128, (j + 1) * 128)
            S = psu.tile([128, N], f32, tag="ps", name=f"S{f}_{c}_{j}")
            for pb, col0 in ((0, 0), (64, 512)):
                nc.tensor.matmul(
                    out=S[:, col0:col0 + 512],
                    lhsT=kTf[pb:pb + 64, js],
                    rhs=qTf[pb:pb + 64, cs],
                    start=True, stop=True,
                )
            Ej = expp.tile([128, N], fp16, tag="e", name=f"E{f}_{c}_{j}")
            nc.scalar.activation(
                out=Ej, in_=S,
                func=mybir.ActivationFunctionType.Exp, scale=0.125)
            if f == 0 and c == 0 and j == 0:
                dump("E000", Ej)
            return Ej

        E_cur = mm2exp(0)
        acc = psu.tile([128, N], f32, tag="ps", name=f"acc{f}_{c}")
        for j in range(JT):
            E_next = mm2exp(j + 1) if j + 1 < JT else None
            for col0, h in ((0, hA), (512, hB)):
                nc.tensor.matmul(
                    out=acc[:, col0:col0 + 512],
                    lhsT=(vaug[j][:, h * DA:(h + 1) * DA]),
                    rhs=(E_cur[:, col0:col0 + 512]),
                    start=(j == 0), stop=(j == JT - 1),
                )
            E_cur = E_next

        if f == 0 and c == 0:
            dump_psum("acc00", acc, f32)
        # custom-DVE ops misbehave at base_partition 64: stage the replicated
        # denominators down to partitions 0-63 with a native copy first
        den = rbp.tile([128, N], f32, tag="den", name=f"den{f}_{c}")
        rb = rbp.tile([128, N], f32, tag="rb", name=f"rb{f}_{c}")
        nc.vector.tensor_copy(out=den[0:64, :], in_=acc[64:128, :])
        nc.vector.reciprocal_approx_fast(out=rb[0:64, :], in_=den[0:64, :])
        if f == 0 and c == 0:
            dump("rb00", rb[:, 0:512])
        for col0, h in ((0, hA), (512, hB)):
            pb = (h % 2) * 64
            nc.vector.tensor_mul(outT[f][pb:pb + 64, cs],
                                 acc[0:64, col0:col0 + 512],
                                 rb[0:64, col0:col0 + 512])

    # ---- main schedule: feat(0), v, then attention with feat(f+1) filler ----
    qTf = qk_feat_tile(0, "q0")
    kTf = qk_feat_tile(E, "k0")
    dump("q0", qTf)
    dump("k0", kTf)
    emit_v_phase()
    for f in range(NE):
        q_next = k_next = None
        attention_chunk(f, 0, qTf, kTf)
        if f + 1 < NE:
            q_next = qk_feat_tile((f + 1) * 128, f"q{f + 1}")
        attention_chunk(f, 1, qTf, kTf)
        if f + 1 < NE:
            k_next = qk_feat_tile(E + (f + 1) * 128, f"k{f + 1}")
        qTf, kTf = q_next, k_next

    dump("outT0", outT[0])

    # ---- proj: yT = w_projh^T @ outT + b  (bf16 x bf16) ----
    for g in range(NE):
        ps_y = psu.tile([128, N], f32, tag="ps", name=f"psy{g}")
        wts = []
        for e in range(NE):
            w = wstr.tile([128, 128], fp16, tag="w", name=f"wp{g}e{e}")
            nc.sync.dma_start(
                out=w, in_=w_projh[e * 128:(e + 1) * 128, g * 128:(g + 1) * 128])
            wts.append(w)
        for c in range(CH):
            cs = slice(c * 512, (c + 1) * 512)
            for e in range(NE):
                nc.tensor.matmul(
                    out=ps_y[:, cs], lhsT=(wts[e]), rhs=(outT[e][:, cs]),
                    start=(e == 0), stop=(e == NE - 1),
                )
        yt = ytp.tile([128, N], f32, tag="yt", name=f"yt{g}")
        nc.vector.tensor_scalar_add(out=yt, in0=ps_y, scalar1=b_sb[:, g:g + 1])
        nc.sync.dma_start(out=yT[g * 128:(g + 1) * 128, :], in_=yt)


def build_nc(loop_n=1, debug_dumps=False):
    """Build + compile the per-core Bass program. loop_n>1 wraps the body in a
    dynamic loop (used only for timing runs)."""
    from contextlib import ExitStack
    import concourse.bacc as bacc
    import concourse.mybir as mybir
    import concourse.tile as tile

    f32 = mybir.dt.float32
    fp16 = mybir.dt.float16
    nc = bacc.Bacc("TRN2", target_bir_lowering=False, debug=False)
    xT = nc.dram_tensor("xTh", [E, N], fp16, kind="ExternalInput").ap()
    w_qkv = nc.dram_tensor("w_qkvh", [E, 3 * E], fp16, kind="ExternalInput").ap()
    w_projh = nc.dram_tensor("w_projh", [E, E], fp16, kind="ExternalInput").ap()
    b_proj = nc.dram_tensor("b_proj", [E], f32, kind="ExternalInput").ap()
    yT = nc.dram_tensor("yT", [E, N], f32, kind="ExternalOutput").ap()

    dbg = None
    if debug_dumps:
        dbg = {
            "vaug0": nc.dram_tensor("d_vaug0", [128, H * DA], fp16,
                                    kind="ExternalOutput").ap(),
            "q0": nc.dram_tensor("d_q0", [128, N], fp16, kind="ExternalOutput").ap(),
            "k0": nc.dram_tensor("d_k0", [128, N], fp16, kind="ExternalOutput").ap(),
            "E000": nc.dram_tensor("d_E000", [128, N], fp16,
                                   kind="ExternalOutput").ap(),
            "acc00": nc.dram_tensor("d_acc00", [128, N], f32,
                                    kind="ExternalOutput").ap(),
            "rb00": nc.dram_tensor("d_rb00", [128, 512], f32,
                                   kind="ExternalOutput").ap(),
            "outT0": nc.dram_tensor("d_outT0", [128, N], fp16,
                                    kind="ExternalOutput").ap(),
        }

    with tile.TileContext(nc) as tc, ExitStack() as ctx:
        pools = (
            ctx.enter_context(tc.tile_pool(name="consts", bufs=1)),
            ctx.enter_context(tc.tile_pool(name="wstr", bufs=12)),
            ctx.enter_context(tc.tile_pool(name="expp", bufs=6)),
            ctx.enter_context(tc.tile_pool(name="qkp", bufs=4)),
            ctx.enter_context(tc.tile_pool(name="rbp", bufs=2)),
            ctx.enter_context(tc.tile_pool(name="ytp", bufs=2)),
            ctx.enter_context(tc.tile_pool(name="psu", bufs=4, space="PSUM")),
        )
        aps = (xT, w_qkv, w_projh, b_proj, yT)
        if loop_n == 1:
            _emit(tc, pools, aps, dbg=dbg)
        else:
            # timing-only path; branch-prefetch hints avoid the per-iteration
            # IRAM refetch stall on the big-body engines
            with tc.For_i(0, loop_n, 1,
                          hint_engines=(mybir.EngineType.PE,
                                        mybir.EngineType.Activation,
                                        mybir.EngineType.DVE)):
                _emit(tc, pools, aps)
    nc.compile()
    return nc


def _get_nc(loop_n=1):
    if loop_n not in _NC_CACHE:
        _NC_CACHE[loop_n] = build_nc(loop_n)
    return _NC_CACHE[loop_n]


def _in_maps(x, w_qkv, w_proj, b_proj):
    x = np.asarray(x, dtype=np.float32)
    w_qkvh = np.ascontiguousarray(
        np.asarray(w_qkv, dtype=np.float32).astype(np.float16))
    w_projh = np.ascontiguousarray(
        np.asarray(w_proj, dtype=np.float32).astype(np.float16))
    b_proj = np.ascontiguousarray(np.asarray(b_proj, dtype=np.float32))
    xTh = np.ascontiguousarray(
        np.transpose(x, (0, 2, 1)).astype(np.float16))  # [B, E, N]
    return [
        {"xTh": xTh[c], "w_qkvh": w_qkvh, "w_projh": w_projh, "b_proj": b_proj}
        for c in range(B)
    ]


def kernel(x, w_qkv, w_proj, b_proj):
    """Full-input entry point: x [8,1024,768] f32 -> out [8,1024,768] f32."""
    from concourse.bass_utils import run_bass_kernel_spmd

    nc = _get_nc()
    in_maps = _in_maps(x, w_qkv, w_proj, b_proj)
    res = run_bass_kernel_spmd(nc, in_maps, core_ids=list(range(B)))
    yT = np.stack([res.results[c]["yT"] for c in range(B)])  # [B, E, N]
    return np.ascontiguousarray(np.transpose(yT, (0, 2, 1)))
